# revision 1
# baseline (speedup 1.0000x reference)
"""BitLinear FFN (BitNet b1.58) Trainium2 kernel, 8-core SPMD.

Strategy: data-parallel over tokens. Every core processes 1024 of the 8192
tokens end-to-end. Weight quantization (+ transpose to contraction-major
layout, + cast to fp8e4) is sharded 1/8 per core and shared via three
pipelined AllGathers (one per matrix). All weights stay cached in SBUF.

Exactness: activation quantization produces integers in [-127, 127] (exact
in bf16) and weight quantization produces {-1, 0, 1} (exact in fp8e4); the
PE accumulates in fp32, so all three matmuls are exact integer arithmetic
(fp8 lhsT x bf16 rhs verified exact on hardware).
Per-token/per-tensor dequant scales are applied on the fly:
  gate = gate_int * c_g          (c_g per token, before silu)
  c_u cancels inside the second act-quant, so `up` stays in integer form
  out  = down_int * F_t          (F_t per token, fused into PSUM evacuation)
"""

import numpy as np

import concourse.bacc as bacc
import concourse.bass as bass
import concourse.bass_isa as bass_isa
import concourse.mybir as mybir
import concourse.tile as tile
from concourse.masks import make_identity

P = 128
HID = 1024
INNER = 4096
N_CORES = 8
T_CORE = 1024          # tokens per core
TC = 256               # token chunk in the main loop
NCH = T_CORE // TC     # 4 chunks
MT = TC // P           # 2 token tiles per chunk
KI = HID // P          # 8 contraction tiles for gate/up
KOG = INNER // P       # 32 contraction tiles for down
OSH = INNER // N_CORES  # 512, o-shard per core
HSH = HID // P         # 8 h-subtiles in w_down shard [1024, 512]

MROUND = 12582912.0    # 1.5 * 2**23: (v + M) - M == round-half-even(v)
W_ELEMS = float(INNER * HID)

F32 = mybir.dt.float32
BF16 = mybir.dt.bfloat16
FP8 = mybir.dt.float8e4

# element offsets inside the per-core staging buffer (bf16 elements)
WG_OFF = 0
WU_OFF = OSH * HID          # 524288
WD_OFF = 2 * OSH * HID      # 1048576
STAGE_ELEMS = 3 * OSH * HID  # 1572864

A = mybir.AluOpType
AF = mybir.ActivationFunctionType


def build_bass(sim_mode: bool = False, main_chunks: int = NCH, reps: int = 1):
    """Build the SPMD program. sim_mode replaces collectives with local
    stand-ins so the single-core cost-model simulator can run it."""
    nc = bacc.Bacc(
        "TRN2", target_bir_lowering=False, debug=False,
        num_devices=N_CORES,
    )
    groups = [list(range(N_CORES))]

    x_d = nc.dram_tensor("x_shard", [T_CORE, HID], F32, kind="ExternalInput")
    wg_d = nc.dram_tensor("wg_shard", [OSH, HID], F32, kind="ExternalInput")
    wu_d = nc.dram_tensor("wu_shard", [OSH, HID], F32, kind="ExternalInput")
    wd_d = nc.dram_tensor("wd_shard", [HID, OSH], F32, kind="ExternalInput")
    out_d = nc.dram_tensor("out_shard", [T_CORE, HID], F32, kind="ExternalOutput")

    wg_r = wg_d.ap().rearrange("(po p) i -> po p i", p=P)    # [4, 128, 1024]
    wu_r = wu_d.ap().rearrange("(po p) i -> po p i", p=P)
    wd_r = wd_d.ap().rearrange("(hs p) o -> hs p o", p=P)    # [8, 128, 512]
    x_r = x_d.ap().rearrange("(n p) i -> n p i", p=P)        # [8, 128, 1024]
    out_r = out_d.ap().rearrange("(n p) h -> n p h", p=P)

    with tile.TileContext(nc) as tc:
        with (
            tc.tile_pool(name="const", bufs=1) as constp,
            tc.tile_pool(name="big", bufs=1) as bigp,
            tc.tile_pool(name="stream", bufs=2) as streamp,
            tc.tile_pool(name="stg", bufs=1) as stgp,
            tc.tile_pool(name="ew", bufs=2) as ewp,
            tc.tile_pool(name="outp", bufs=2) as outpp,
            tc.tile_pool(name="tiny", bufs=2) as tinyp,
            tc.tile_pool(name="pg", bufs=4, space="PSUM") as pgp,
            tc.tile_pool(name="pd", bufs=2, space="PSUM") as pdp,
            tc.tile_pool(name="pt", bufs=2, space="PSUM") as ptp,
            tc.tile_pool(name="dram", bufs=1, space="DRAM") as dramp,
        ):
            ident = constp.tile([P, P], BF16)
            make_identity(nc, ident)
            ones_col = constp.tile([P, 1], F32)
            nc.gpsimd.memset(ones_col[:], 1.0)

            def emit_body():
                # ---------------- |w| partial sums over this core's shards -----
                sums_col = constp.tile([P, 4], F32)
                nc.gpsimd.memset(sums_col[:], 0.0)
                for j, (src, n_sub) in enumerate(
                    ((wg_r, 4), (wu_r, 4), (wd_r, HSH))
                ):
                    for po in range(n_sub):
                        wld = streamp.tile([P, HID], F32, tag="wld")
                        nc.sync.dma_start(out=wld[:, :src.shape[2]], in_=src[po])
                        part = tinyp.tile([P, 1], F32, tag="wabs")
                        nc.vector.tensor_reduce(
                            out=part[:], in_=wld[:, :src.shape[2]],
                            axis=mybir.AxisListType.X,
                            op=A.add, apply_absolute_value=True)
                        nc.vector.tensor_tensor(
                            out=sums_col[:, j:j + 1], in0=sums_col[:, j:j + 1],
                            in1=part[:], op=A.add)

                psums = ptp.tile([1, 4], F32, tag="pt")
                nc.tensor.matmul(psums[:], lhsT=ones_col[:], rhs=sums_col[:],
                                 start=True, stop=True)
                sums_sb = tinyp.tile([1, 4], F32)
                nc.vector.tensor_copy(out=sums_sb[:], in_=psums[:])

                # tiny AllReduce of the three |w| sums
                sums_in = dramp.tile([1, 4], F32)
                sums_out = dramp.tile([1, 4], F32, addr_space="Shared")
                nc.sync.dma_start(out=sums_in[:], in_=sums_sb[:])
                if sim_mode:
                    nc.sync.dma_start(out=sums_out[:], in_=sums_in[:])
                else:
                    nc.gpsimd.collective_compute(
                        "AllReduce", A.add, replica_groups=groups,
                        ins=[sums_in[:]], outs=[sums_out[:]])
                sums_all = tinyp.tile([1, 4], F32)
                nc.sync.dma_start(out=sums_all[:], in_=sums_out[:])

                # clip-means (= 1/s_w) and s_w, broadcast to all partitions
                mcl = tinyp.tile([1, 4], F32)
                nc.vector.tensor_scalar(
                    out=mcl[:], in0=sums_all[:], scalar1=1.0 / W_ELEMS,
                    scalar2=1e-5, op0=A.mult, op1=A.max)
                sw = tinyp.tile([1, 4], F32)
                nc.vector.reciprocal(out=sw[:], in_=mcl[:])
                swb = constp.tile([P, 4], F32)
                nc.gpsimd.partition_broadcast(swb[:], sw[0:1, :])
                mclb = constp.tile([P, 4], F32)
                nc.gpsimd.partition_broadcast(mclb[:], mcl[0:1, :])
                # bc_ud = clip_mean_wu * clip_mean_wd / 127^2   (for F_t)
                bc_ud = constp.tile([P, 1], F32)
                nc.vector.tensor_tensor(
                    out=bc_ud[:], in0=mclb[:, 1:2], in1=mclb[:, 2:3], op=A.mult)
                nc.vector.tensor_scalar_mul(bc_ud[:], bc_ud[:], 1.0 / (127.0 * 127.0))

                # ---------------- quantize + transpose weight shards -----------
                shared_as = "Local" if sim_mode else "Shared"
                stg_wg = dramp.tile([OSH * HID], FP8)
                stg_wu = dramp.tile([OSH * HID], FP8)
                stg_wd = dramp.tile([OSH * HID], FP8)
                agt_wg = dramp.tile([N_CORES, OSH * HID], FP8, addr_space=shared_as)
                agt_wu = dramp.tile([N_CORES, OSH * HID], FP8, addr_space=shared_as)
                ag_wd = dramp.tile([N_CORES, OSH * HID], FP8, addr_space=shared_as)

                def do_ag(stg, ag):
                    if sim_mode:
                        for c in range(N_CORES):
                            nc.sync.dma_start(out=ag[c, :], in_=stg[:])
                    else:
                        nc.gpsimd.collective_compute(
                            "AllGather", A.bypass, replica_groups=groups,
                            ins=[stg[:]], outs=[ag[:]])

                def quant_tile(src_sb, w_idx, width):
                    """bf16 tile <- clip(round(src * s_w), -1, 1); src clobbered."""
                    sw_col = swb[:, w_idx:w_idx + 1]
                    nc.vector.tensor_scalar(
                        out=src_sb, in0=src_sb, scalar1=sw_col, scalar2=MROUND,
                        op0=A.mult, op1=A.add)
                    wq_b = streamp.tile([P, HID], BF16, tag="wqb")
                    nc.vector.tensor_scalar(
                        out=wq_b[:, :width], in0=src_sb, scalar1=-MROUND,
                        scalar2=1.0, op0=A.add, op1=A.min)
                    nc.vector.tensor_scalar_max(
                        wq_b[:, :width], wq_b[:, :width], -1.0)
                    return wq_b

                # gate / up: stage layout [KI, 128 i, OSH o]
                for src, stg, ag, w_idx in (
                    (wg_r, stg_wg, agt_wg, 0), (wu_r, stg_wu, agt_wu, 1),
                ):
                    stage_sb = stgp.tile([P, KI, OSH], FP8, tag="stg")
                    for po in range(4):
                        wld = streamp.tile([P, HID], F32, tag="wld")
                        nc.sync.dma_start(out=wld[:], in_=src[po])
                        wq_b = quant_tile(wld[:], w_idx, HID)
                        for ki in range(KI):
                            pt_t = ptp.tile([P, P], BF16, tag="pt")
                            nc.tensor.transpose(
                                pt_t[:], wq_b[:, ki * P:(ki + 1) * P], ident[:])
                            nc.vector.tensor_copy(
                                out=stage_sb[:, ki, po * P:(po + 1) * P], in_=pt_t[:])
                    nc.sync.dma_start(
                        out=stg[:].rearrange(
                            "(ki i o) -> i ki o", ki=KI, i=P, o=OSH),
                        in_=stage_sb[:])
                    do_ag(stg, ag)

                # down: stage layout [4 ko, 128 o, HID h]
                stage_sb = stgp.tile([P, 4, HID], FP8, tag="stg")
                for hs in range(HSH):
                    wld = streamp.tile([P, HID], F32, tag="wld")
                    nc.sync.dma_start(out=wld[:, :OSH], in_=wd_r[hs])
                    wq_b = quant_tile(wld[:, :OSH], 2, OSH)
                    for ko in range(4):
                        pt_t = ptp.tile([P, P], BF16, tag="pt")
                        nc.tensor.transpose(
                            pt_t[:], wq_b[:, ko * P:(ko + 1) * P], ident[:])
                        nc.vector.tensor_copy(
                            out=stage_sb[:, ko, hs * P:(hs + 1) * P], in_=pt_t[:])
                nc.sync.dma_start(
                    out=stg_wd[:].rearrange(
                        "(ko o h) -> o ko h", ko=4, o=P, h=HID),
                    in_=stage_sb[:])
                do_ag(stg_wd, ag_wd)

                ag_wg = [
                    agt_wg[c, :].rearrange(
                        "(ki i o) -> i ki o", ki=KI, i=P, o=OSH)
                    for c in range(N_CORES)
                ]
                ag_wu = [
                    agt_wu[c, :].rearrange(
                        "(ki i o) -> i ki o", ki=KI, i=P, o=OSH)
                    for c in range(N_CORES)
                ]

                # ---------------- x shard: quantize + transpose -----------------
                xqT = bigp.tile([P, KI, T_CORE], BF16, tag="xqT")   # [i, t]
                absm_c = constp.tile([P, KI], F32)                  # clip(absmax_x)

                for ts in range(KI):  # 8 token tiles of 128
                    x_sb = streamp.tile([P, HID], F32, tag="xld")
                    nc.sync.dma_start(out=x_sb[:], in_=x_r[ts])
                    am = tinyp.tile([P, 1], F32, tag="am")
                    nc.vector.tensor_reduce(
                        out=am[:], in_=x_sb[:], axis=mybir.AxisListType.X,
                        op=A.max, apply_absolute_value=True)
                    nc.vector.tensor_scalar_max(absm_c[:, ts:ts + 1], am[:], 1e-5)
                    s1c = tinyp.tile([P, 1], F32, tag="s1c")
                    nc.vector.reciprocal(out=s1c[:], in_=absm_c[:, ts:ts + 1])
                    nc.vector.tensor_scalar_mul(s1c[:], s1c[:], 127.0)
                    # xq = round(x * s1) in-place then cast
                    nc.vector.tensor_scalar(
                        out=x_sb[:], in0=x_sb[:], scalar1=s1c[:, 0:1],
                        scalar2=MROUND, op0=A.mult, op1=A.add)
                    xq_b = streamp.tile([P, HID], BF16, tag="wqb")
                    nc.vector.tensor_scalar(
                        out=xq_b[:], in0=x_sb[:], scalar1=-MROUND, scalar2=None,
                        op0=A.add)
                    for ki in range(KI):
                        pt_t = ptp.tile([P, P], BF16, tag="pt")
                        nc.tensor.transpose(
                            pt_t[:], xq_b[:, ki * P:(ki + 1) * P], ident[:])
                        nc.vector.tensor_copy(
                            out=xqT[:, ki, ts * P:(ts + 1) * P], in_=pt_t[:])

                # c_g: clip(absmax_x) * clip_mean_wg / 127, as [1, T] row
                cg_col = constp.tile([P, KI], F32)
                nc.vector.tensor_scalar(
                    out=cg_col[:], in0=absm_c[:], scalar1=mclb[:, 0:1],
                    scalar2=1.0 / 127.0, op0=A.mult, op1=A.mult)
                cg_row = constp.tile([1, T_CORE], F32)
                for ts in range(KI):
                    nc.sync.dma_start(
                        out=cg_row[0:1, ts * P:(ts + 1) * P],
                        in_=cg_col[:, ts:ts + 1])

                # ---------------- cached transposed w_down ---------------------
                wdt4 = bigp.tile([P, N_CORES, 4, HID], FP8, tag="wdt")  # [o, c, ko, h]
                for c in range(N_CORES):
                    nc.sync.dma_start(
                        out=wdt4[:, c],
                        in_=ag_wd[c, :].rearrange(
                            "(ko o h) -> o ko h", ko=4, o=P, h=HID))
                wdt = wdt4[:].rearrange("o c ko h -> o (c ko) h")

                # ---------------- cached transposed gate/up weights -------------
                wg_sb = bigp.tile([P, KI, INNER], FP8, tag="wgc")
                wu_sb = bigp.tile([P, KI, INNER], FP8, tag="wuc")
                for c in range(N_CORES):
                    csl = slice(c * OSH, (c + 1) * OSH)
                    nc.sync.dma_start(out=wg_sb[:, :, csl], in_=ag_wg[c])
                    nc.sync.dma_start(out=wu_sb[:, :, csl], in_=ag_wu[c])

                # ---------------- main loop over token chunks -------------------
                for ch in range(main_chunks):
                    tsl = slice(ch * TC, (ch + 1) * TC)
                    cgb = ewp.tile([P, TC], F32, tag="cgb")
                    nc.gpsimd.partition_broadcast(cgb[:], cg_row[0:1, tsl])

                    prod = bigp.tile([P, KOG, TC], F32, tag="prod")
                    for m in range(KOG):
                        osl = slice(m * P, (m + 1) * P)
                        psg = pgp.tile([P, TC], F32, tag="pg")
                        for ki in range(KI):
                            nc.tensor.matmul(
                                psg[:], lhsT=wg_sb[:, ki, osl], rhs=xqT[:, ki, tsl],
                                start=(ki == 0), stop=(ki == KI - 1))
                        psu = pgp.tile([P, TC], F32, tag="pg")
                        for ki in range(KI):
                            nc.tensor.matmul(
                                psu[:], lhsT=wu_sb[:, ki, osl], rhs=xqT[:, ki, tsl],
                                start=(ki == 0), stop=(ki == KI - 1))
                        # gate*c_g -> silu -> * up_int
                        gsc = ewp.tile([P, TC], F32, tag="gsc")
                        nc.vector.tensor_tensor(
                            out=gsc[:], in0=psg[:], in1=cgb[:], op=A.mult)
                        gsil = ewp.tile([P, TC], F32, tag="gsil")
                        nc.scalar.activation(gsil[:], gsc[:], AF.Silu)
                        nc.vector.tensor_tensor(
                            out=prod[:, m], in0=gsil[:], in1=psu[:], op=A.mult)

                    # second act-quant: absmax over o (free sub-dim + partitions)
                    # 4-way split over kog so most reduces overlap the m-loop
                    nparts = 4
                    kq = KOG // nparts
                    maxr = ewp.tile([P, TC], F32, tag="maxr", bufs=1)
                    mr2 = ewp.tile([P, TC], F32, tag="maxr2", bufs=1)
                    for q in range(nparts):
                        dst = maxr if q == 0 else mr2
                        nc.vector.tensor_reduce(
                            out=dst[:],
                            in_=prod[:, q * kq:(q + 1) * kq].rearrange(
                                "p k t -> p t k"),
                            axis=mybir.AxisListType.X, op=A.max,
                            apply_absolute_value=True)
                        if q > 0:
                            nc.vector.tensor_tensor(
                                out=maxr[:], in0=maxr[:], in1=mr2[:], op=A.max)
                    maxg = ewp.tile([P, TC], F32, tag="maxg")
                    nc.gpsimd.partition_all_reduce(
                        maxg[:], maxr[:], channels=P,
                        reduce_op=bass_isa.ReduceOp.max)
                    nc.vector.tensor_scalar_max(maxg[:], maxg[:], 1e-5)
                    s2b = ewp.tile([P, TC], F32, tag="s2b")
                    nc.vector.reciprocal(out=s2b[:], in_=maxg[:])
                    nc.vector.tensor_scalar_mul(s2b[:], s2b[:], 127.0)

                    # quantize per k-slice so down matmuls consume them streaming
                    prodq = bigp.tile([P, KOG, TC], BF16, tag="prodq")
                    for k in range(0, KOG, 2):
                        nc.vector.tensor_tensor(
                            out=prod[:, k:k + 2], in0=prod[:, k:k + 2],
                            in1=s2b[:, None, :].to_broadcast((P, 2, TC)),
                            op=A.mult)
                        nc.vector.tensor_scalar(
                            out=prodq[:, k:k + 2], in0=prod[:, k:k + 2],
                            scalar1=MROUND, scalar2=-MROUND,
                            op0=A.add, op1=A.add)

                    # F_t column form for this chunk
                    fcol = tinyp.tile([P, MT], F32, tag="fcol")
                    for mt in range(MT):
                        nc.sync.dma_start(
                            out=fcol[:, mt:mt + 1],
                            in_=maxg[0:1, mt * P:(mt + 1) * P])
                    nc.vector.tensor_tensor(
                        out=fcol[:], in0=fcol[:],
                        in1=absm_c[:, ch * MT:(ch + 1) * MT], op=A.mult)
                    nc.vector.tensor_scalar_mul(fcol[:], fcol[:], bc_ud[:, 0:1])

                    # down projection
                    for mt in range(MT):
                        t0 = mt * P
                        for hh in range(2):
                            hsl = slice(hh * 512, (hh + 1) * 512)
                            psd = pdp.tile([P, 512], F32, tag="pd")
                            for kog in range(KOG):
                                nc.tensor.matmul(
                                    psd[:], lhsT=prodq[:, kog, t0:t0 + P],
                                    rhs=wdt[:, kog, hsl],
                                    start=(kog == 0), stop=(kog == KOG - 1))
                            osb = outpp.tile([P, 512], F32, tag="osb")
                            nc.scalar.activation(
                                osb[:], psd[:], AF.Copy, scale=fcol[:, mt:mt + 1])
                            nc.sync.dma_start(
                                out=out_r[ch * MT + mt][:, hsl], in_=osb[:])


            for _rep in range(reps):
                emit_body()

    nc.compile()
    return nc


_NC_CACHE = {}


def _get_nc():
    if "nc" not in _NC_CACHE:
        _NC_CACHE["nc"] = build_bass(sim_mode=False)
    return _NC_CACHE["nc"]


def make_in_maps(x, w_gate, w_up, w_down):
    x2 = np.ascontiguousarray(
        np.asarray(x, dtype=np.float32).reshape(N_CORES * T_CORE, HID))
    wg = np.asarray(w_gate, dtype=np.float32)
    wu = np.asarray(w_up, dtype=np.float32)
    wd = np.asarray(w_down, dtype=np.float32)
    in_maps = []
    for c in range(N_CORES):
        in_maps.append({
            "x_shard": np.ascontiguousarray(x2[c * T_CORE:(c + 1) * T_CORE]),
            "wg_shard": np.ascontiguousarray(wg[c * OSH:(c + 1) * OSH]),
            "wu_shard": np.ascontiguousarray(wu[c * OSH:(c + 1) * OSH]),
            "wd_shard": np.ascontiguousarray(wd[:, c * OSH:(c + 1) * OSH]),
        })
    return in_maps


def assemble_output(results):
    parts = [results[c]["out_shard"] for c in range(N_CORES)]
    return np.concatenate(parts, axis=0).reshape(4, 2048, HID)


def kernel(x, w_gate, w_up, w_down):
    from concourse.bass_utils import run_bass_kernel_spmd
    nc = _get_nc()
    in_maps = make_in_maps(x, w_gate, w_up, w_down)
    res = run_bass_kernel_spmd(nc, in_maps, list(range(N_CORES)), trace=False)
    return assemble_output(res.results)



# revision 4
# speedup vs baseline: 4.9345x; 4.9345x over previous
"""BitLinear FFN (BitNet b1.58) Trainium2 kernel, 8-core SPMD — v2.

Strategy: data-parallel over tokens (1024 tokens/core). Host passes
pre-transposed inputs (xT, wgT, wuT, wdT) so no PE transposes are needed.
Weight quantization is sharded 1/8 per core (two streamed passes: |w| sums,
then quantize->fp8 into AllGather staging). Three pipelined AllGathers
(wg -> wu -> wd) overlap the main loop: chunk-0 gate matmuls start once wg
lands; the up phase covers the wu AG; the down phase covers the wd AG.

Exactness: act quant produces ints in [-127,127] (exact bf16); weights are
ternary fp8e4. PE accumulates fp32 -> integer-exact matmuls. The silu'd
gate and the gate*up product are carried in fp16 (rel err ~2^-11, well under
the 2e-2 gate). Scales:
  gate = gate_int * c_g  (c_g = absmax_x * mean|wg| / 127, per token)
  c_u cancels inside the second act-quant; `up` stays integer
  out  = down_int * F_t  (F_t = maxg * absmax_x * mean|wu| mean|wd| / 127^2)
"""

import numpy as np

import concourse.bacc as bacc
import concourse.bass as bass
import concourse.bass_isa as bass_isa
import concourse.mybir as mybir
import concourse.tile as tile

P = 128
HID = 1024
INNER = 4096
N_CORES = 8
T_CORE = 1024          # tokens per core
TC = 256               # token chunk in the main loop
NCH = T_CORE // TC     # 4 chunks
MT = TC // P           # 2 token tiles per chunk
KI = HID // P          # 8 contraction tiles for gate/up
KOG = INNER // P       # 32 contraction tiles for down
OSH = INNER // N_CORES  # 512, o-shard per core

MROUND = 12582912.0    # 1.5 * 2**23: (v + M) - M == round-half-even(v)
W_ELEMS = float(INNER * HID)

F32 = mybir.dt.float32
BF16 = mybir.dt.bfloat16
FP16 = mybir.dt.float16
FP8 = mybir.dt.float8e4

A = mybir.AluOpType
AF = mybir.ActivationFunctionType


def build_bass(sim_mode: bool = False, main_chunks: int = NCH, reps: int = 1):
    """Build the SPMD program. sim_mode replaces collectives with local
    stand-ins so the single-core cost-model simulator can run it."""
    nc = bacc.Bacc(
        "TRN2", target_bir_lowering=False, debug=False,
        num_devices=N_CORES,
    )
    groups = [list(range(N_CORES))]

    xT_d = nc.dram_tensor("xT_shard", [HID, T_CORE], F32, kind="ExternalInput")
    wg_d = nc.dram_tensor("wgT_shard", [HID, OSH], F32, kind="ExternalInput")
    wu_d = nc.dram_tensor("wuT_shard", [HID, OSH], F32, kind="ExternalInput")
    wd_d = nc.dram_tensor("wdT_shard", [OSH, HID], F32, kind="ExternalInput")
    out_d = nc.dram_tensor("out_shard", [T_CORE, HID], F32, kind="ExternalOutput")

    xT_r = xT_d.ap().rearrange("(ki p) t -> ki p t", p=P)     # [8, 128, 1024]
    wg_r = wg_d.ap().rearrange("(ki p) o -> ki p o", p=P)     # [8, 128, 512]
    wu_r = wu_d.ap().rearrange("(ki p) o -> ki p o", p=P)
    wd_r = wd_d.ap().rearrange("(ko p) h -> ko p h", p=P)     # [4, 128, 1024]
    out_r = out_d.ap().rearrange("(n p) h -> n p h", p=P)     # [8, 128, 1024]

    with tile.TileContext(nc) as tc:
        with (
            tc.tile_pool(name="const", bufs=1) as constp,
            tc.tile_pool(name="big", bufs=1) as bigp,
            tc.tile_pool(name="wstr", bufs=4) as wstrp,
            tc.tile_pool(name="xstr", bufs=2) as xstrp,
            tc.tile_pool(name="stg", bufs=1) as stgp,
            tc.tile_pool(name="ew", bufs=2) as ewp,
            tc.tile_pool(name="outp", bufs=2) as outpp,
            tc.tile_pool(name="tiny", bufs=2) as tinyp,
            tc.tile_pool(name="pg", bufs=4, space="PSUM") as pgp,
            tc.tile_pool(name="pd", bufs=2, space="PSUM") as pdp,
            tc.tile_pool(name="pt", bufs=2, space="PSUM") as ptp,
            tc.tile_pool(name="dram", bufs=1, space="DRAM") as dramp,
        ):
            ones_col = constp.tile([P, 1], F32)
            nc.gpsimd.memset(ones_col[:], 1.0)

            def emit_body():
                dmaq = [nc.sync, nc.scalar]

                # ---------- pass 1: |w| partial sums over this core's shards
                sums_col = constp.tile([P, 4], F32)
                nc.gpsimd.memset(sums_col[:], 0.0)
                for j, (src, n_sub, w_, tag) in enumerate((
                    (wg_r, KI, OSH, "ws512"), (wu_r, KI, OSH, "ws512"),
                    (wd_r, 4, HID, "ws1024"),
                )):
                    for t in range(n_sub):
                        wld = wstrp.tile([P, w_], F32, tag=tag, bufs=3)
                        dmaq[t % 2].dma_start(out=wld[:], in_=src[t])
                        part = tinyp.tile([P, 1], F32, tag="wabs")
                        nc.vector.tensor_reduce(
                            out=part[:], in_=wld[:],
                            axis=mybir.AxisListType.X,
                            op=A.add, apply_absolute_value=True)
                        nc.vector.tensor_tensor(
                            out=sums_col[:, j:j + 1], in0=sums_col[:, j:j + 1],
                            in1=part[:], op=A.add)

                psums = ptp.tile([1, 4], F32, tag="pt")
                nc.tensor.matmul(psums[:], lhsT=ones_col[:], rhs=sums_col[:],
                                 start=True, stop=True)
                sums_sb = tinyp.tile([1, 4], F32)
                nc.vector.tensor_copy(out=sums_sb[:], in_=psums[:])

                # tiny AllReduce of the three |w| sums
                sums_in = dramp.tile([1, 4], F32)
                sums_out = dramp.tile([1, 4], F32, addr_space="Shared")
                nc.sync.dma_start(out=sums_in[:], in_=sums_sb[:])
                if sim_mode:
                    nc.sync.dma_start(out=sums_out[:], in_=sums_in[:])
                else:
                    nc.gpsimd.collective_compute(
                        "AllReduce", A.add, replica_groups=groups,
                        ins=[sums_in[:]], outs=[sums_out[:]])
                sums_all = tinyp.tile([1, 4], F32)
                nc.sync.dma_start(out=sums_all[:], in_=sums_out[:])

                # ---------- x pass 1: per-token absmax via partition reduce
                amr = constp.tile([P, T_CORE], F32)     # clip(absmax_x), bcast
                for ts in range(KI):
                    xld = xstrp.tile([P, T_CORE], F32, tag="xld")
                    dmaq[ts % 2].dma_start(out=xld[:], in_=xT_r[ts])
                    nc.scalar.activation(xld[:], xld[:], AF.Abs)
                    nc.vector.tensor_tensor(
                        out=amr[:], in0=(xld if ts == 0 else amr)[:],
                        in1=xld[:], op=A.max)
                nc.gpsimd.partition_all_reduce(
                    amr[:], amr[:], channels=P,
                    reduce_op=bass_isa.ReduceOp.max)
                nc.vector.tensor_scalar_max(amr[:], amr[:], 1e-5)
                s1b = ewp.tile([P, T_CORE], F32, tag="s1b", bufs=1)
                nc.vector.reciprocal(out=s1b[:], in_=amr[:])
                nc.vector.tensor_scalar_mul(s1b[:], s1b[:], 127.0)
                # absmax as [token-partition, tile] columns (for F_t)
                absm_c = constp.tile([P, KI], F32)
                for ts in range(KI):
                    nc.gpsimd.dma_start(
                        out=absm_c[:, ts:ts + 1],
                        in_=amr[0:1, ts * P:(ts + 1) * P])

                # ---------- scales from the AllReduced sums
                mcl = tinyp.tile([1, 4], F32)
                nc.vector.tensor_scalar(
                    out=mcl[:], in0=sums_all[:], scalar1=1.0 / W_ELEMS,
                    scalar2=1e-5, op0=A.mult, op1=A.max)
                sw = tinyp.tile([1, 4], F32)
                nc.vector.reciprocal(out=sw[:], in_=mcl[:])
                swb = constp.tile([P, 4], F32)
                nc.gpsimd.partition_broadcast(swb[:], sw[0:1, :])
                mclb = constp.tile([P, 4], F32)
                nc.gpsimd.partition_broadcast(mclb[:], mcl[0:1, :])
                # bc_ud = clip_mean_wu * clip_mean_wd / 127^2   (for F_t)
                bc_ud = constp.tile([P, 1], F32)
                nc.vector.tensor_tensor(
                    out=bc_ud[:], in0=mclb[:, 1:2], in1=mclb[:, 2:3], op=A.mult)
                nc.vector.tensor_scalar_mul(bc_ud[:], bc_ud[:], 1.0 / (127.0 * 127.0))

                # ---------- pass 2: quantize weight shards -> fp8 staging -> AG
                shared_as = "Local" if sim_mode else "Shared"
                stg_wg = dramp.tile([KI * P * OSH], FP8)
                stg_wu = dramp.tile([KI * P * OSH], FP8)
                stg_wd = dramp.tile([4 * P * HID], FP8)
                agt_wg = dramp.tile([N_CORES, KI * P * OSH], FP8, addr_space=shared_as)
                agt_wu = dramp.tile([N_CORES, KI * P * OSH], FP8, addr_space=shared_as)
                agt_wd = dramp.tile([N_CORES, 4 * P * HID], FP8, addr_space=shared_as)

                def do_ag(stg, ag):
                    if sim_mode:
                        for c in range(N_CORES):
                            dmaq[c % 2].dma_start(out=ag[c, :], in_=stg[:])
                    else:
                        nc.gpsimd.collective_compute(
                            "AllGather", A.bypass, replica_groups=groups,
                            ins=[stg[:]], outs=[ag[:]])

                def quant_tile(src_sb, dst_sb, w_idx, width):
                    """dst fp8 <- clip(round(src * s_w), -1, 1); src clobbered."""
                    sw_col = swb[:, w_idx:w_idx + 1]
                    nc.vector.tensor_scalar(
                        out=src_sb, in0=src_sb, scalar1=sw_col, scalar2=MROUND,
                        op0=A.mult, op1=A.add)
                    nc.vector.tensor_scalar(
                        out=src_sb, in0=src_sb, scalar1=-MROUND,
                        scalar2=1.0, op0=A.add, op1=A.min)
                    nc.vector.tensor_scalar(
                        out=dst_sb, in0=src_sb, scalar1=-1.0, scalar2=None,
                        op0=A.max)

                # wg first (gates the main loop), then x requant, wu, wd
                stage_g = stgp.tile([P, KI, OSH], FP8, tag="stg8", bufs=1)
                for t in range(KI):
                    wld = wstrp.tile([P, OSH], F32, tag="ws512", bufs=3)
                    dmaq[t % 2].dma_start(out=wld[:], in_=wg_r[t])
                    quant_tile(wld[:], stage_g[:, t, :], 0, OSH)
                nc.sync.dma_start(
                    out=stg_wg[:].rearrange(
                        "(ki p o) -> p ki o", ki=KI, p=P, o=OSH),
                    in_=stage_g[:])
                do_ag(stg_wg, agt_wg)

                # x pass 2: quantize xT -> xqT (bf16 ints)
                xqT = bigp.tile([P, KI, T_CORE], BF16, tag="xqT")
                for ts in range(KI):
                    xld = xstrp.tile([P, T_CORE], F32, tag="xld")
                    dmaq[ts % 2].dma_start(out=xld[:], in_=xT_r[ts])
                    nc.vector.tensor_tensor(
                        out=xld[:], in0=xld[:], in1=s1b[:], op=A.mult)
                    nc.vector.tensor_scalar(
                        out=xqT[:, ts, :], in0=xld[:], scalar1=MROUND,
                        scalar2=-MROUND, op0=A.add, op1=A.add)

                stage_u = stgp.tile([P, KI, OSH], FP8, tag="stg8", bufs=1)
                for t in range(KI):
                    wld = wstrp.tile([P, OSH], F32, tag="ws512", bufs=3)
                    dmaq[t % 2].dma_start(out=wld[:], in_=wu_r[t])
                    quant_tile(wld[:], stage_u[:, t, :], 1, OSH)
                nc.sync.dma_start(
                    out=stg_wu[:].rearrange(
                        "(ki p o) -> p ki o", ki=KI, p=P, o=OSH),
                    in_=stage_u[:])
                do_ag(stg_wu, agt_wu)

                stage_d = stgp.tile([P, 4, HID], FP8, tag="stgd")
                for t in range(4):
                    wld = wstrp.tile([P, HID], F32, tag="ws1024", bufs=3)
                    dmaq[t % 2].dma_start(out=wld[:], in_=wd_r[t])
                    quant_tile(wld[:], stage_d[:, t, :], 2, HID)
                nc.sync.dma_start(
                    out=stg_wd[:].rearrange(
                        "(ko p h) -> p ko h", ko=4, p=P, h=HID),
                    in_=stage_d[:])
                do_ag(stg_wd, agt_wd)

                # ---------- load gathered weights into SBUF caches
                wg_sb = bigp.tile([P, KI, INNER], FP8, tag="wgc")
                wu_sb = bigp.tile([P, KI, INNER], FP8, tag="wuc")
                wdt = bigp.tile([P, KOG, HID], FP8, tag="wdc")
                for c in range(N_CORES):
                    csl = slice(c * OSH, (c + 1) * OSH)
                    dmaq[c % 2].dma_start(
                        out=wg_sb[:, :, csl],
                        in_=agt_wg[c, :].rearrange(
                            "(ki p o) -> p ki o", ki=KI, p=P, o=OSH))
                for c in range(N_CORES):
                    csl = slice(c * OSH, (c + 1) * OSH)
                    dmaq[c % 2].dma_start(
                        out=wu_sb[:, :, csl],
                        in_=agt_wu[c, :].rearrange(
                            "(ki p o) -> p ki o", ki=KI, p=P, o=OSH))
                for c in range(N_CORES):
                    dmaq[c % 2].dma_start(
                        out=wdt[:, c * 4:(c + 1) * 4, :],
                        in_=agt_wd[c, :].rearrange(
                            "(ko p h) -> p ko h", ko=4, p=P, h=HID))

                # ---------- main loop over token chunks (sw-pipelined) -------
                prods = [
                    bigp.tile([P, KOG, TC], FP16, tag=f"prod{i}",
                              name=f"prod{i}")
                    for i in range(2)
                ]

                def gu_phase(ch):
                    tsl = slice(ch * TC, (ch + 1) * TC)
                    prod = prods[ch % 2]
                    # c_g broadcast tile for this chunk
                    cgb = ewp.tile([P, TC], F32, tag="cgb")
                    nc.vector.tensor_scalar(
                        out=cgb[:], in0=amr[:, tsl], scalar1=mclb[:, 0:1],
                        scalar2=1.0 / 127.0, op0=A.mult, op1=A.mult)

                    # gate: silu(gate_int * c_g) -> prod (fp16)
                    for m in range(KOG):
                        osl = slice(m * P, (m + 1) * P)
                        psg = pgp.tile([P, TC], F32, tag="pg")
                        for ki in range(KI):
                            nc.tensor.matmul(
                                psg[:], lhsT=wg_sb[:, ki, osl],
                                rhs=xqT[:, ki, tsl],
                                start=(ki == 0), stop=(ki == KI - 1))
                        nc.vector.tensor_tensor(
                            out=psg[:], in0=psg[:], in1=cgb[:], op=A.mult)
                        nc.scalar.activation(prod[:, m], psg[:], AF.Silu)

                    # up: prod *= up_int; running per-token absmax
                    maxr = ewp.tile([P, TC], F32, tag="maxr")
                    for m in range(KOG):
                        osl = slice(m * P, (m + 1) * P)
                        psu = pgp.tile([P, TC], F32, tag="pg")
                        for ki in range(KI):
                            nc.tensor.matmul(
                                psu[:], lhsT=wu_sb[:, ki, osl],
                                rhs=xqT[:, ki, tsl],
                                start=(ki == 0), stop=(ki == KI - 1))
                        nc.vector.tensor_tensor(
                            out=prod[:, m], in0=prod[:, m], in1=psu[:],
                            op=A.mult)
                        pab = ewp.tile([P, TC], FP16, tag="pab")
                        nc.scalar.activation(pab[:], prod[:, m], AF.Abs)
                        nc.vector.tensor_tensor(
                            out=maxr[:], in0=(pab if m == 0 else maxr)[:],
                            in1=pab[:], op=A.max)

                    # second act-quant scale
                    maxg = ewp.tile([P, TC], F32, tag="maxg")
                    nc.gpsimd.partition_all_reduce(
                        maxg[:], maxr[:], channels=P,
                        reduce_op=bass_isa.ReduceOp.max)
                    nc.vector.tensor_scalar_max(maxg[:], maxg[:], 1e-5)
                    s2b = ewp.tile([P, TC], F32, tag="s2b")
                    nc.vector.reciprocal(out=s2b[:], in_=maxg[:])
                    nc.vector.tensor_scalar_mul(s2b[:], s2b[:], 127.0)

                    # quantize prod in place (fp16 ints in [-127, 127])
                    for g in range(0, KOG, 2):
                        qt = ewp.tile([P, 2, TC], F32, tag="qtmp", bufs=1)
                        nc.vector.tensor_tensor(
                            out=qt[:], in0=prod[:, g:g + 2],
                            in1=s2b[:, None, :].to_broadcast((P, 2, TC)),
                            op=A.mult)
                        nc.vector.tensor_scalar(
                            out=prod[:, g:g + 2], in0=qt[:], scalar1=MROUND,
                            scalar2=-MROUND, op0=A.add, op1=A.add)

                    # F_t column form for this chunk
                    fcol = tinyp.tile([P, MT], F32, tag="fcol")
                    for mt in range(MT):
                        nc.gpsimd.dma_start(
                            out=fcol[:, mt:mt + 1],
                            in_=maxg[0:1, mt * P:(mt + 1) * P])
                    nc.vector.tensor_tensor(
                        out=fcol[:], in0=fcol[:],
                        in1=absm_c[:, ch * MT:(ch + 1) * MT], op=A.mult)
                    nc.vector.tensor_scalar_mul(fcol[:], fcol[:], bc_ud[:, 0:1])
                    return fcol

                def down_phase(ch, fcol):
                    prod = prods[ch % 2]
                    for mt in range(MT):
                        t0 = mt * P
                        for hh in range(2):
                            hsl = slice(hh * 512, (hh + 1) * 512)
                            psd = pdp.tile([P, 512], F32, tag="pd")
                            for kog in range(KOG):
                                nc.tensor.matmul(
                                    psd[:], lhsT=prod[:, kog, t0:t0 + P],
                                    rhs=wdt[:, kog, hsl],
                                    start=(kog == 0), stop=(kog == KOG - 1))
                            osb = outpp.tile([P, 512], F32, tag="osb")
                            nc.scalar.activation(
                                osb[:], psd[:], AF.Copy,
                                scale=fcol[:, mt:mt + 1])
                            nc.sync.dma_start(
                                out=out_r[ch * MT + mt][:, hsl], in_=osb[:])

                fcols = {}
                for ch in range(main_chunks):
                    fcols[ch] = gu_phase(ch)
                    if ch > 0:
                        down_phase(ch - 1, fcols[ch - 1])
                if main_chunks > 0:
                    down_phase(main_chunks - 1, fcols[main_chunks - 1])

            for _rep in range(reps):
                emit_body()

    nc.compile()
    return nc


_NC_CACHE = {}


def _get_nc():
    if "nc" not in _NC_CACHE:
        _NC_CACHE["nc"] = build_bass(sim_mode=False)
    return _NC_CACHE["nc"]


def make_in_maps(x, w_gate, w_up, w_down):
    x2 = np.asarray(x, dtype=np.float32).reshape(N_CORES * T_CORE, HID)
    wg = np.asarray(w_gate, dtype=np.float32)
    wu = np.asarray(w_up, dtype=np.float32)
    wd = np.asarray(w_down, dtype=np.float32)
    in_maps = []
    for c in range(N_CORES):
        in_maps.append({
            "xT_shard": np.ascontiguousarray(
                x2[c * T_CORE:(c + 1) * T_CORE].T),
            "wgT_shard": np.ascontiguousarray(
                wg[c * OSH:(c + 1) * OSH].T),
            "wuT_shard": np.ascontiguousarray(
                wu[c * OSH:(c + 1) * OSH].T),
            "wdT_shard": np.ascontiguousarray(
                wd[:, c * OSH:(c + 1) * OSH].T),
        })
    return in_maps


def assemble_output(results):
    parts = [results[c]["out_shard"] for c in range(N_CORES)]
    return np.concatenate(parts, axis=0).reshape(4, 2048, HID)


def kernel(x, w_gate, w_up, w_down):
    from concourse.bass_utils import run_bass_kernel_spmd
    nc = _get_nc()
    in_maps = make_in_maps(x, w_gate, w_up, w_down)
    res = run_bass_kernel_spmd(nc, in_maps, list(range(N_CORES)), trace=False)
    return assemble_output(res.results)
